# revision 1
# baseline (speedup 1.0000x reference)
"""Trainium2 Bass kernel for a 3-layer FCL + size-5 sliding-window stack.

Reference computation (fp32):
    h = relu(x @ W1.T)          # [N, 10]
    t = relu(h @ W2.T + b2)     # [N, 5]
    out[n] = concat(t[n-2..n+2])  zero-padded  -> [N, 25]

Strategy (8 cores, data-parallel over rows, halo recomputed per core):
  - Each core gets a 25088-row shard (25000 own rows + 2-row halo each side,
    zero padded, rounded up to 49 blocks of 512 rows).
  - Per 512-row block: x is DMA-loaded with a fused f32->bf16 cast (SWDGE),
    rows (4p+j) on partition p so the HBM read is 5120 contiguous bytes per
    partition.  bf16 tiles are transposed on the tensor engine so the
    320-dim contraction sits on partitions (bf16 streams 1 col/cycle vs 2
    for fp32); L1 accumulates hT[10,512] in fp32 PSUM; L2 multiplies back
    to natural layout t[128, 4x5]; bias on DVE, relu on ACT.
  - t rows stream to a DRAM scratch t_buf[25088, 5] (fp32).  Output row n is
    the contiguous 25-element window t_buf.flat[5n:5n+25]: each partition
    DMA-loads one contiguous 8-row strip (160B descriptors), DVE expands the
    five overlapping windows via a strided read AP, and the result stores
    with 400B-per-partition descriptors.  Gathers lag two blocks so their
    waits are pre-satisfied and never stall the queues.
  - The ISA allows ONE sync-wait per instruction and Tile does not split
    multi-waits: engines are choreographed so nearly every instruction has
    at most one unobserved producer, and a post-pass hoists any remaining
    extra waits onto same-engine NoOps.
  - Global zero-padding of the window (4 tiny row slices at the ends of the
    full output) is patched on the host during unsharding.
"""

import numpy as np

import bass_rust
import concourse.bass as bass
import concourse.mybir as mybir
import concourse.tile as tile
from concourse.tile import add_dep_helper

# ---- problem constants (hardcoded per contract) ----
N = 200000
D = 320
D1 = 10
D2 = 5
W = 5
HALF = W // 2
NCORES = 8
ROWS = N // NCORES          # 25000 output rows per core
BLK = 512                   # rows per compute block
JJ = 4                      # rows per partition per block (512 = 128*4)
NBLK = 49                   # ceil((ROWS + 4) / BLK) -> 25088 padded rows
PAD = NBLK * BLK            # 25088
CHUNKS = [(0, 128), (128, 128), (256, 64)]  # d-chunks of 320
F32 = mybir.dt.float32
BF16 = mybir.dt.bfloat16
RELU = mybir.ActivationFunctionType.Relu

_NC_CACHE = {}


def _dep(a, b, why):
    add_dep_helper(a.ins, b.ins, reason=why)


def split_multiwaits(nc):
    """Walrus/ISA allows ONE sync-wait per instruction; Tile emits several.

    For every instruction with >1 wait, hoist all but the last wait onto
    fresh NoOps on the same engine immediately before it.  The engine
    stalls at the nops exactly as it would have at the instruction, so
    semantics are unchanged.
    """
    n_split = 0
    for bb in nc.main_func.blocks:
        insts = bb.instructions
        out = []
        changed = False
        for ins in insts:
            si = ins.sync_info
            waits = list(si.on_wait) if si is not None else []
            if len(waits) > 1:
                changed = True
                for w in waits[:-1]:
                    n_split += 1
                    nop = bass_rust.InstNoOp(name=f"wsplit-{n_split}")
                    nop.engine = ins.engine
                    nop.sync_info = bass_rust.SyncInfo(
                        on_wait=[w], on_update=[]
                    )
                    nc.inst_map[nop.name] = nop
                    out.append(nop)
                ins.sync_info = bass_rust.SyncInfo(
                    on_wait=[waits[-1]], on_update=list(si.on_update)
                )
            out.append(ins)
        if changed:
            bb.instructions = out
    return n_split


def build_nc():
    nc = bass.Bass("TRN2", target_bir_lowering=False, debug=False)

    x_t = nc.dram_tensor("x", [PAD, D], F32, kind="ExternalInput")
    w1_t = nc.dram_tensor("W1", [D1, D], F32, kind="ExternalInput")
    w2_t = nc.dram_tensor("W2", [D2, D1], F32, kind="ExternalInput")
    b2_t = nc.dram_tensor("b2", [D2], F32, kind="ExternalInput")
    out_t = nc.dram_tensor("out", [ROWS, W * D2], F32, kind="ExternalOutput")
    tbuf_t = nc.dram_tensor("t_buf", [PAD, D2], F32)  # internal scratch

    with tile.TileContext(nc) as tc:
        with (
            tc.tile_pool(name="singles", bufs=1) as singles,
            tc.tile_pool(name="xpool", bufs=6) as xpool,
            tc.tile_pool(name="xtpool", bufs=16) as xtpool,
            tc.tile_pool(name="hpool", bufs=8) as hpool,
            tc.tile_pool(name="tpool", bufs=NBLK) as tpool,
            tc.tile_pool(name="wpool", bufs=NBLK) as wpool,
            tc.tile_pool(name="gpool", bufs=NBLK) as gpool,
            tc.tile_pool(name="ps_xt", bufs=4, space="PSUM") as ps_xt,
            tc.tile_pool(name="ps_h", bufs=2, space="PSUM") as ps_h,
            tc.tile_pool(name="ps_t", bufs=2, space="PSUM") as ps_t,
        ):
            # ---- constants ----
            ident = singles.tile([128, 128], BF16)
            nc.gpsimd.memset(ident, 0.0)
            asel = nc.gpsimd.affine_select(
                out=ident,
                in_=ident,
                compare_op=mybir.AluOpType.not_equal,
                fill=1.0,
                base=0,
                pattern=[[-1, 128]],
                channel_multiplier=1,
            )
            w1_sb = singles.tile([D1, D], F32)
            nc.sync.dma_start(out=w1_sb, in_=w1_t[:, :])
            w2_sb = singles.tile([D2, D1], F32)
            nc.sync.dma_start(out=w2_sb, in_=w2_t[:, :])
            # b2 replicated across partitions and the 4 row-subtiles
            b2rep = singles.tile([128, JJ, D2], F32)
            b2dma = nc.gpsimd.dma_start(
                out=b2rep, in_=bass.AP(b2_t, 0, [[0, 128], [0, JJ], [1, D2]])
            )

            # bf16 casts of the weights (DVE), then PE transposes.
            w1_bf = singles.tile([D1, D], BF16)
            nc.vector.tensor_copy(out=w1_bf, in_=w1_sb)
            w2_bf = singles.tile([D2, D1], BF16)
            nc.vector.tensor_copy(out=w2_bf, in_=w2_sb)

            # PE observes the identity build once; transposes then only wait
            # on their data producer.
            nop_id = nc.tensor.nop()
            _dep(nop_id, asel, "PE pre-observe identity")

            w2t_sb = singles.tile([D1, D2], BF16)
            wps = ps_xt.tile([128, BLK], F32, tag="xt", name="wps_w2")
            nc.tensor.matmul(wps[:D1, :D2], w2_bf[:, :], ident[:D2, :D2], start=True, stop=True)
            nc.scalar.copy(out=w2t_sb, in_=wps[:D1, :D2])

            w1t_sb = singles.tile([128, len(CHUNKS), D1], BF16)
            last_wcopy = None
            for c, (d0, cw) in enumerate(CHUNKS):
                wps = ps_xt.tile([128, BLK], F32, tag="xt", name=f"wps_{c}")
                nc.tensor.matmul(
                    wps[:cw, :D1],
                    w1_bf[:, d0 : d0 + cw],
                    ident[:D1, :D1],
                    start=True,
                    stop=True,
                )
                last_wcopy = nc.scalar.copy(
                    out=w1t_sb[:cw, c, :], in_=wps[:cw, :D1]
                )
            # PE observes the weight copies (ACT) once.
            nop_w = nc.tensor.nop()
            _dep(nop_w, last_wcopy, "PE pre-observe W1T/W2T copies")
            # DVE observes the b2 broadcast once.
            nop_b2 = nc.vector.nop()
            _dep(nop_b2, b2dma, "DVE pre-observe b2 broadcast")

            stores = {}
            gwins = {}
            gready = {}

            def emit_gather_load(g):
                """t_buf -> SBUF strips + DVE window expansion.

                Partition p loads the contiguous 8 rows [512g+4p, 512g+4p+8)
                (one 160B descriptor per partition); window w of output row
                512g+4p+j is strip elements [5(j+w), 5(j+w)+5) - an affine
                overlapping read the DVE expands into [128, JJ, 25].
                RAW deps are stores g and g+1 (two DMAHW lanes): a sync nop
                observes store g so the DMA itself waits only on store g+1.
                """
                nrows = min(BLK, ROWS - BLK * g)
                npart = nrows // JJ
                nop_g = nc.sync.nop()
                _dep(nop_g, stores[g], "SP pre-observe t store g")
                win_sb = wpool.tile([128, (JJ + W - 1) * D2], F32, tag="w")
                nc.sync.dma_start(
                    out=win_sb[:npart],
                    in_=bass.AP(
                        tbuf_t,
                        BLK * g * D2,
                        [[JJ * D2, npart], [1, (JJ + W - 1) * D2]],
                    ),
                )
                g_sb = gpool.tile([128, JJ, W * D2], F32, tag="g")
                last = None
                for j in range(JJ):
                    last = nc.vector.tensor_copy(
                        out=g_sb[:npart, j, :],
                        in_=bass.AP(
                            win_sb.tensor,
                            win_sb.offset + j * D2,
                            [[win_sb.ap[0][0], npart], [D2, W], [1, D2]],
                        ),
                    )
                gwins[g] = (g_sb, npart)
                gready[g] = last

            def emit_gather_store(g):
                g_sb, npart = gwins[g]
                nc.sync.dma_start(
                    out=bass.AP(
                        out_t,
                        BLK * g * W * D2,
                        [[JJ * W * D2, npart], [W * D2, JJ], [1, W * D2]],
                    ),
                    in_=g_sb[:npart],
                )

            def emit_tail(b, h_sbs):
                """L2 + bias/relu + t store for block b (lagged one block
                so the L2 matmuls never stall the PE queue on a fresh relu)."""
                h_sb = h_sbs[b]
                t_ps = ps_t.tile([128, JJ, D2], F32, tag="t")
                for j in range(JJ):
                    nc.tensor.matmul(
                        t_ps[:, j, :],
                        h_sb[:, j * 128 : (j + 1) * 128],
                        w2t_sb,
                        start=True,
                        stop=True,
                    )
                t_sb = tpool.tile([128, JJ, D2], F32, tag="ts")
                nc.vector.tensor_add(t_sb, t_ps, b2rep)
                nc.scalar.activation(t_sb, t_sb, RELU)
                stores[b] = nc.sync.dma_start(
                    out=bass.AP(
                        tbuf_t, b * BLK * D2, [[JJ * D2, 128], [D2, JJ], [1, D2]]
                    ),
                    in_=t_sb,
                )

            # ---- main loop over 512-row blocks (software-pipelined) ----
            h_sbs = {}
            for b in range(NBLK):
                # rows [512b, 512b+512): partition p holds rows 4p+j as
                # contiguous 5120B reads, cast f32->bf16 in the DMA (SWDGE).
                # rotate tags explicitly: guarantees round-robin slot reuse
                # (6-block WAR distance) even if the pool free-list is LIFO
                x_sb = xpool.tile([128, JJ, D], BF16, tag=f"x{b % 6}")
                for half in range(2):
                    nc.gpsimd.dma_start(
                        out=x_sb[:, 2 * half : 2 * half + 2, :],
                        in_=bass.AP(
                            x_t,
                            b * BLK * D + half * 2 * D,
                            [[JJ * D, 128], [D, 2], [1, D]],
                        ),
                    )

                # all 12 transposes first: the PE FIFO never blocks on a
                # PSUM->SBUF copy while transposes are still runnable.
                xt_pss = []
                xt_sbs = []
                for c, (d0, cw) in enumerate(CHUNKS):
                    xt_ps = ps_xt.tile([128, BLK], BF16, tag="xt")
                    xt_sb = xtpool.tile([128, BLK], BF16, tag="xts")
                    for j in range(JJ):
                        nc.tensor.transpose(
                            xt_ps[:cw, j * 128 : (j + 1) * 128],
                            x_sb[:, j, d0 : d0 + cw],
                            ident,
                        )
                    if c == 1:
                        nc.vector.tensor_copy(out=xt_sb[:cw], in_=xt_ps[:cw])
                    else:
                        nc.scalar.copy(out=xt_sb[:cw], in_=xt_ps[:cw])
                    xt_pss.append(xt_ps)
                    xt_sbs.append(xt_sb)

                h_ps = ps_h.tile([D1, BLK], F32, tag="h")
                for c, (d0, cw) in enumerate(CHUNKS):
                    nc.tensor.matmul(
                        h_ps,
                        w1t_sb[:cw, c, :],
                        xt_sbs[c][:cw],
                        start=(c == 0),
                        stop=(c == len(CHUNKS) - 1),
                    )

                h_sb = hpool.tile([D1, BLK], BF16, tag="hs")
                nc.scalar.activation(h_sb, h_ps, RELU)
                h_sbs[b] = h_sb

                if b >= 1:
                    emit_tail(b - 1, h_sbs)
                if b >= 3:
                    emit_gather_load(b - 3)
                if b >= 4:
                    emit_gather_store(b - 4)

            emit_tail(NBLK - 1, h_sbs)
            for g in (NBLK - 3, NBLK - 2, NBLK - 1):
                emit_gather_load(g)
            for g in (NBLK - 4, NBLK - 3, NBLK - 2, NBLK - 1):
                emit_gather_store(g)

    split_multiwaits(nc)
    return nc


def make_shards(x):
    """Per-core [PAD, D] shards with +-2 halo rows, zero padded."""
    shards = []
    for c in range(NCORES):
        s = np.zeros((PAD, D), dtype=np.float32)
        lo = ROWS * c - HALF
        hi = ROWS * c + ROWS + HALF
        src_lo, src_hi = max(lo, 0), min(hi, N)
        s[src_lo - lo : src_lo - lo + (src_hi - src_lo)] = x[src_lo:src_hi]
        shards.append(s)
    return shards


def _patch_edges(out):
    # the reference zero-pads t, not x: window slots that fall outside
    # [0, N) must be exactly zero.
    out[0, : 2 * D2] = 0.0
    out[1, :D2] = 0.0
    out[N - 2, 4 * D2 :] = 0.0
    out[N - 1, 3 * D2 :] = 0.0
    return out


def run(inputs, trace=False):
    from concourse.bass_utils import run_bass_kernel_spmd

    x = np.ascontiguousarray(np.asarray(inputs["x"], dtype=np.float32))
    W1 = np.ascontiguousarray(np.asarray(inputs["W1"], dtype=np.float32))
    W2 = np.ascontiguousarray(np.asarray(inputs["W2"], dtype=np.float32))
    b2 = np.ascontiguousarray(np.asarray(inputs["b2"], dtype=np.float32))
    assert x.shape == (N, D)

    if "nc" not in _NC_CACHE:
        _NC_CACHE["nc"] = build_nc()
    nc = _NC_CACHE["nc"]

    in_maps = [{"x": s, "W1": W1, "W2": W2, "b2": b2} for s in make_shards(x)]
    res = run_bass_kernel_spmd(nc, in_maps, list(range(NCORES)), trace=trace)
    out = np.concatenate([res.results[c]["out"] for c in range(NCORES)], axis=0)
    return _patch_edges(out), res


def kernel(**inputs):
    out, _ = run(inputs, trace=False)
    return out



# revision 2
# speedup vs baseline: 1.5492x; 1.5492x over previous
"""Trainium2 Bass kernel for a 3-layer FCL + size-5 sliding-window stack.

Reference computation (fp32):
    h = relu(x @ W1.T)          # [N, 10]
    t = relu(h @ W2.T + b2)     # [N, 5]
    out[n] = concat(t[n-2..n+2])  zero-padded  -> [N, 25]

Strategy (8 cores, data-parallel over rows, halo recomputed per core):
  - Host prep is layout-only: x is cast to bf16 and pre-transposed so each
    core receives xT [320, 25088] (25000 own rows + 2-row halo each side,
    zero padded).  This halves the HBM x read (16MB vs 32MB fp32) and puts
    the 320-dim contraction directly on partitions - the tensor engine
    never has to transpose anything.
  - The whole pipeline runs in the transposed layout:
      L1: hT[10,512] = w1t_chunk.T @ xT_chunk   (3 chunk matmuls, K=128/128/64)
      DVE: h = relu(hT) cast to bf16
      L2: tT[5,512] = w2t.T @ h                 (K=10)
      ACT: tT_all[:, cols] = relu(tT + b2)      (bias is per-partition!)
    tT_all [5, 25088] f32 lives entirely in SBUF (~98KB/partition) - no
    DRAM round trip for t.
  - The size-5 window gather costs nothing: outT[5w+c, n] = tT_all[c, n+w],
    so the store DMA's affine AP does the 5x window replication straight
    out of SBUF (25 descriptors of 8KB per 2048-row superblock).
  - x loads stream on the SP HWDGE ring (3 DMAs of 512KB per superblock,
    triple buffered); stores go on the ACT ring so they never head-of-line
    block a load.
  - Host unshard: concat the per-core outT [25, 25000] along columns,
    transpose to [200000, 25], patch the 4 global-edge window slots to
    exact zero (the reference zero-pads t, not x).
  - The ISA allows ONE sync-wait per instruction; a post-pass hoists any
    extra waits onto same-engine NoOps.
"""

import numpy as np
import ml_dtypes

import bass_rust
import concourse.bass as bass
import concourse.mybir as mybir
import concourse.tile as tile

# ---- problem constants (hardcoded per contract) ----
N = 200000
D = 320
D1 = 10
D2 = 5
W = 5
HALF = W // 2
NCORES = 8
ROWS = N // NCORES          # 25000 output rows per core
BLK = 512                   # rows per compute block (one PSUM bank)
NBLK = 49                   # 25088 padded rows of t per core
PAD = NBLK * BLK            # 25088
SBLK = 4                    # compute blocks per superblock (DMA granularity)
CHUNKS = [(0, 128), (128, 128), (256, 64)]  # d-chunks of 320
F32 = mybir.dt.float32
BF16 = mybir.dt.bfloat16
RELU = mybir.ActivationFunctionType.Relu
BF = ml_dtypes.bfloat16

_NC_CACHE = {}


def split_multiwaits(nc):
    """Walrus/ISA allows ONE sync-wait per instruction; Tile emits several.

    For every instruction with >1 wait, hoist all but the last wait onto
    fresh NoOps on the same engine immediately before it.  The engine
    stalls at the nops exactly as it would have at the instruction, so
    semantics are unchanged.
    """
    n_split = 0
    for bb in nc.main_func.blocks:
        insts = bb.instructions
        out = []
        changed = False
        for ins in insts:
            si = ins.sync_info
            waits = list(si.on_wait) if si is not None else []
            if len(waits) > 1:
                changed = True
                for w in waits[:-1]:
                    n_split += 1
                    nop = bass_rust.InstNoOp(name=f"wsplit-{n_split}")
                    nop.engine = ins.engine
                    nop.sync_info = bass_rust.SyncInfo(
                        on_wait=[w], on_update=[]
                    )
                    nc.inst_map[nop.name] = nop
                    out.append(nop)
                ins.sync_info = bass_rust.SyncInfo(
                    on_wait=[waits[-1]], on_update=list(si.on_update)
                )
            out.append(ins)
        if changed:
            bb.instructions = out
    return n_split


def build_nc():
    nc = bass.Bass("TRN2", target_bir_lowering=False, debug=False)

    xT_t = nc.dram_tensor("xT", [D, PAD], BF16, kind="ExternalInput")
    w1t_t = nc.dram_tensor("W1T", [D, D1], BF16, kind="ExternalInput")
    w2t_t = nc.dram_tensor("W2T", [D1, D2], BF16, kind="ExternalInput")
    b2_t = nc.dram_tensor("b2", [D2], F32, kind="ExternalInput")
    out_t = nc.dram_tensor("outT", [W * D2, ROWS], F32, kind="ExternalOutput")

    # superblock start columns (in t rows): 12 x 2048 + 1 x 512
    sb_starts = list(range(0, PAD, SBLK * BLK))
    sb_lens = [min(SBLK * BLK, PAD - s) for s in sb_starts]
    NSB = len(sb_starts)

    with tile.TileContext(nc) as tc:
        with (
            tc.tile_pool(name="singles", bufs=1) as singles,
            tc.tile_pool(name="xpool", bufs=3) as xpool,
            tc.tile_pool(name="hpool", bufs=4) as hpool,
            tc.tile_pool(name="ps_h", bufs=2, space="PSUM") as ps_h,
            tc.tile_pool(name="ps_t", bufs=2, space="PSUM") as ps_t,
        ):
            # ---- constants (one-time) ----
            w1t_sb = singles.tile([128, len(CHUNKS), D1], BF16)
            for c, (d0, cw) in enumerate(CHUNKS):
                nc.sync.dma_start(
                    out=w1t_sb[:cw, c, :],
                    in_=bass.AP(w1t_t, d0 * D1, [[D1, cw], [1, D1]]),
                )
            w2t_sb = singles.tile([D1, D2], BF16)
            nc.sync.dma_start(out=w2t_sb, in_=w2t_t[:, :])
            b2_sb = singles.tile([D2, 1], F32)
            nc.sync.dma_start(
                out=b2_sb, in_=bass.AP(b2_t, 0, [[1, D2], [1, 1]])
            )
            # persistent t.T accumulator [5, 25088] f32 (~98KB/partition)
            tT_all = singles.tile([D2, PAD], F32)

            x_sbs = {}      # sb index -> list of 3 chunk tiles
            h_sbs = {}      # block index -> h tile [10, 512] bf16
            t_pss = {}      # block index -> tT psum tile [5, 512]

            def emit_loads(s):
                tiles = []
                for c, (d0, cw) in enumerate(CHUNKS):
                    xt = xpool.tile([128, SBLK * BLK], BF16, tag=f"x{c}")
                    nc.sync.dma_start(
                        out=xt[:cw, : sb_lens[s]],
                        in_=bass.AP(
                            xT_t,
                            d0 * PAD + sb_starts[s],
                            [[PAD, cw], [1, sb_lens[s]]],
                        ),
                    )
                    tiles.append(xt)
                x_sbs[s] = tiles

            def emit_l1(b):
                """3 chunk matmuls + DVE relu for block b."""
                s, r = divmod(b, SBLK)
                h_ps = ps_h.tile([D1, BLK], F32, tag="h")
                for c, (d0, cw) in enumerate(CHUNKS):
                    nc.tensor.matmul(
                        h_ps,
                        w1t_sb[:cw, c, :],
                        x_sbs[s][c][:cw, r * BLK : (r + 1) * BLK],
                        start=(c == 0),
                        stop=(c == len(CHUNKS) - 1),
                    )
                h_sb = hpool.tile([D1, BLK], BF16, tag="hs")
                nc.vector.tensor_scalar_max(h_sb, h_ps, 0.0)
                h_sbs[b] = h_sb

            def emit_l2(b):
                """L2 matmul for block b (lagged one block so the PE never
                stalls on a fresh DVE relu)."""
                t_ps = ps_t.tile([D2, BLK], F32, tag="t")
                nc.tensor.matmul(
                    t_ps, w2t_sb, h_sbs[b], start=True, stop=True
                )
                t_pss[b] = t_ps

            def emit_bias_relu(b):
                """ACT: tT_all[:, block cols] = relu(tT_ps + b2)."""
                nc.scalar.activation(
                    tT_all[:, b * BLK : (b + 1) * BLK],
                    t_pss[b],
                    RELU,
                    bias=b2_sb,
                )
                del t_pss[b]

            def emit_store(s):
                """outT[5w+c, n] = tT_all[c, n+w] for this superblock's n.
                The affine AP does the 5x window replication from SBUF."""
                n0 = sb_starts[s]
                ln = min(sb_lens[s], ROWS - n0)
                nc.scalar.dma_start(
                    out=bass.AP(
                        out_t,
                        n0,
                        [[ROWS, D2], [D2 * ROWS, W], [1, ln]],
                    ),
                    in_=bass.AP(
                        tT_all.tensor,
                        tT_all.offset + n0,
                        [[tT_all.ap[0][0], D2], [1, W], [1, ln]],
                    ),
                )

            # ---- main loop (software-pipelined) ----
            emit_loads(0)
            emit_loads(1)
            for b in range(NBLK):
                s, r = divmod(b, SBLK)
                if r == 0 and s + 2 < NSB:
                    emit_loads(s + 2)
                emit_l1(b)
                if b >= 1:
                    emit_l2(b - 1)
                if b >= 2:
                    emit_bias_relu(b - 2)
                # store superblock s-1 once its +3 halo cols exist
                # (after bias_relu of block 4s, i.e. when b-2 == 4s)
                if r == 2 and s >= 1:
                    emit_store(s - 1)

            emit_l2(NBLK - 1)
            emit_bias_relu(NBLK - 2)
            emit_bias_relu(NBLK - 1)
            emit_store(NSB - 2)
            emit_store(NSB - 1)

    split_multiwaits(nc)
    return nc


def make_shards(x):
    """Per-core xT [320, PAD] bf16 shards with +-2 col halo, zero padded."""
    xbT = np.ascontiguousarray(x.astype(BF).T)  # [320, N]
    shards = []
    for c in range(NCORES):
        s = np.zeros((D, PAD), dtype=BF)
        lo = ROWS * c - HALF
        src_lo, src_hi = max(lo, 0), min(lo + PAD, N)
        s[:, src_lo - lo : src_lo - lo + (src_hi - src_lo)] = xbT[
            :, src_lo:src_hi
        ]
        shards.append(s)
    return shards


def _patch_edges(out):
    # the reference zero-pads t, not x: window slots that fall outside
    # [0, N) must be exactly zero.
    out[0, : 2 * D2] = 0.0
    out[1, :D2] = 0.0
    out[N - 2, 4 * D2 :] = 0.0
    out[N - 1, 3 * D2 :] = 0.0
    return out


def run(inputs, trace=False):
    from concourse.bass_utils import run_bass_kernel_spmd

    x = np.ascontiguousarray(np.asarray(inputs["x"], dtype=np.float32))
    W1 = np.asarray(inputs["W1"], dtype=np.float32)
    W2 = np.asarray(inputs["W2"], dtype=np.float32)
    b2 = np.ascontiguousarray(np.asarray(inputs["b2"], dtype=np.float32))
    assert x.shape == (N, D)

    W1T = np.ascontiguousarray(W1.T).astype(BF)
    W2T = np.ascontiguousarray(W2.T).astype(BF)

    if "nc" not in _NC_CACHE:
        _NC_CACHE["nc"] = build_nc()
    nc = _NC_CACHE["nc"]

    in_maps = [
        {"xT": s, "W1T": W1T, "W2T": W2T, "b2": b2} for s in make_shards(x)
    ]
    res = run_bass_kernel_spmd(nc, in_maps, list(range(NCORES)), trace=trace)
    out = np.ascontiguousarray(
        np.concatenate(
            [res.results[c]["outT"] for c in range(NCORES)], axis=1
        ).T
    )
    return _patch_edges(out), res


def kernel(**inputs):
    out, _ = run(inputs, trace=False)
    return out
